# revision 25
# baseline (speedup 1.0000x reference)
"""Trainium2 Bass kernel for nn_FLAttention (sparse_attention) — j-layout.

Math (per batch b, head h), q = aq*x+bq, k = ak*x+bk, v = av*x+bv:
  S[i,j] = 1/(|k_j - q_i| + eps);  P = softmax_j(S);  att = P v / sqrt(H)
  out = x + sum_h att

Key idea vs the i-layout baseline: put KEYS j on partitions and queries i
on the free dim.  Then Z_i = sum_j p_ji and N_i = sum_j p_ji*avx_j and
X_i = sum_j p_ji*x_j are PE matmuls (lhsT = [avx; 1; x], M=3) instead of
Pool/DVE elementwise passes.  Softmax stability without a row max: clamp
a = max(|d|, 1/C) with C=160 and bias p = exp(r - 85); all terms live in
[e^-85, e^75] (fp32/bf16 safe) and ratios below the clamp are exact.
Rows where the clamp ties multiple keys are repaired in the epilogue:
for tied rows  sum_j p (v_j - (av/ak) d_j)/Z = v(d=0)  identically, so
att_corr = att - rho*(av/ak)*(ak*X/Z + u)  with rho = [Z > e^75.2].

Per 128x1024 tile: PE d-matmul (K=2, f32r, two N=512 chunks into a
2-deep PSUM rotation), DVE custom op CLAMP_RECIP (abs+clamp+1NR recip,
8 ALU stages, fp16 out), ACT exp batched EXPB tiles/instr (bf16 p), PE
value matmul (M=3 = [avISH*x; 1; x] columns, bf16, PSUM-accumulated at
partition offsets 32g).  TRN2 notes: GPSIMD/Pool cannot touch PSUM and
has no divide/pow; compute operands need 32-aligned partition bases.
Epilogue: val tiles -> SBUF via ACT Identity (PSUM-capable), strided
gathers of N/Z/X rows, the v*-correction chain for the small-ak heads
(ordered into val tile 0) on the otherwise-idle Pool overlapped with
the main pipeline, a short DVE tail (reciprocal + N*iZ - corr), one
K=8 bf16 matmul summing heads, and an exact fp32 x+bvs residual add.

Sharding: data-parallel over batch: B=16 -> 2 batches per core, 8 cores.
"""
import numpy as np
import ml_dtypes

import concourse.bass as bass
import concourse.bacc as bacc
import concourse.mybir as mybir
import concourse.tile as tile
from concourse.bass_utils import run_bass_kernel_spmd

B, D, H = 16, 1024, 4
N_CORES = 8
BPC = B // N_CORES          # batches per core
NPAIR = BPC * H             # (b,h) pairs per core
NT = D // 128               # j-tiles per pair

F32 = mybir.dt.float32
F32R = mybir.dt.float32r
BF16 = mybir.dt.bfloat16
FP16 = mybir.dt.float16
OP = mybir.AluOpType
AF = mybir.ActivationFunctionType

CLAMP = 160.0               # s-clamp: a = max(|d|, 1/CLAMP)
BIAS = 85.0                 # p = exp(r - BIAS)
THETA = float(np.exp(75.2))  # correction trigger: Z > THETA
ISH = float(1.0 / np.sqrt(np.float32(H)))
# pair order: correction heads (0,1) of both batches first (-> val tile 0),
# then heads 2,3 (-> val tile 1, no epilogue correction needed)
PERM = [(0, 0), (0, 1), (1, 0), (1, 1), (0, 2), (0, 3), (1, 2), (1, 3)]
EXPB = 2                    # tiles per batched exp instruction
R4B = 6                     # r4 pool depth
P4B = 6                     # p4 pool depth
VLAG = 2                    # val-matmul group lag

# (pair, jt) tiles whose abs+clamp runs on Pool (divide then on DVE):
# tiles whose transform goes ACT-abs -> Pool (max, then pow -1):
POW_TILES = frozenset((p, jt) for p in range(8) for jt in (2, 6)
                      if (p, jt) not in ((0, 2), (1, 2), (0, 6), (1, 6),
                                         (2, 2), (3, 2)))

# ---------------- custom DVE op: r = NR1(1/max(|d|, C2)) --------------------
from concourse.dve_spec import (Spec, Src0, C0, C1, C2, Zero, Bin, AluOp,
                                 lower)
from concourse.dve_uop import DveOpSpec
from concourse.dve_ops import DveOp, RECIP_APPROX_FAST_CONSTS
import concourse.dve_ops as dve_ops

RECIP_NAME = "CLAMP_RECIP_ANT"
C0V = RECIP_APPROX_FAST_CONSTS["s0"]
C1V = RECIP_APPROX_FAST_CONSTS["s1"]


def _clamp_recip_ref(in0, in1, c0, c1, c2):
    # a = max(d, -d, c2); 1-NR reciprocal from BITWISE_NOT exponent seed
    x = np.maximum(in0.astype(np.float32),
                   (np.float32(0.0) - in0).astype(np.float32))
    a = np.maximum(x, np.float32(c2))
    not_a = (~a.view(np.int32)).view(np.float32)
    y0 = not_a * np.float32(c0)
    y1 = (y0 * (np.float32(c1) - a * y0)).astype(np.float32)
    return y1


def _register_recip_op():
    if RECIP_NAME in dve_ops._SUB_OPCODE_FOR_NAME:
        for o in dve_ops.OPS:
            if o.name == RECIP_NAME:
                return o
    t = Bin(AluOp.SUBTRACT, Zero, Src0)
    x = Bin(AluOp.MAX, Src0, t)
    a = Bin(AluOp.MAX, x, C2)
    nx = Bin(AluOp.BITWISE_NOT, a, a)
    y0 = Bin(AluOp.MULTIPLY, nx, C0)
    y1 = Bin(AluOp.MULTIPLY, y0,
             Bin(AluOp.SUBTRACT, C1, Bin(AluOp.MULTIPLY, a, y0)))
    spec = Spec(body=y1, reference=_clamp_recip_ref)
    row = max(dve_ops._SUB_OPCODE_FOR_NAME.values()) + 1
    assert row < 0x20
    dve_ops._SUB_OPCODE_FOR_NAME[RECIP_NAME] = row
    shas = {}
    for ver in ("v3", "v4"):
        s = DveOpSpec(name=RECIP_NAME, opcode=row, uops=lower(spec, ver=ver),
                      rd1_en=False)
        shas[ver] = s.sha(ver)
    op = DveOp(RECIP_NAME, spec, subdim=False, uops_sha=shas)
    dve_ops.OPS.append(op)
    dve_ops.CUSTOM_DVE_SPECS[RECIP_NAME] = spec
    return op


RECIP_OP = _register_recip_op()


def build_bass():
    nc = bacc.Bacc(
        "TRN2",
        target_bir_lowering=False,
        debug=False,
        enable_asserts=False,
        num_devices=N_CORES,
    )
    # host-prepped inputs (see kernel()):
    # xr  = [f32r-rounded x rows (BPC); ones row]
    # u8  = per-pair rows cc_h - aq_h*x_b (f32r)   [also correction input]
    # akr = f32r ak values
    # avx = per-pair bf16 rows av_h*ISH*x_b ; xbf = bf16 x rows
    # col8 = [ak(8); cav(8)] per-pair columns; lhy = y-matmul lhsT [11,2]
    xr_d = nc.dram_tensor("xr", (BPC + 1, D), F32, kind="ExternalInput").ap()
    u8_d = nc.dram_tensor("u8", (NPAIR, D), F32, kind="ExternalInput").ap()
    akb_d = nc.dram_tensor("akb", (NPAIR, D), F32, kind="ExternalInput").ap()
    avx_d = nc.dram_tensor("avx", (NPAIR, D), BF16, kind="ExternalInput").ap()
    xbf8_d = nc.dram_tensor("xbf8", (NPAIR, D), BF16,
                            kind="ExternalInput").ap()
    col8_d = nc.dram_tensor("col8", (8, 2), F32, kind="ExternalInput").ap()
    lhy_d = nc.dram_tensor("lhy", (8, 2), BF16, kind="ExternalInput").ap()
    u8h_d = nc.dram_tensor("u8h", (NPAIR, D), BF16,
                           kind="ExternalInput").ap()
    xb2_d = nc.dram_tensor("xb2", (BPC, D), F32,
                           kind="ExternalInput").ap()
    crit_d = nc.dram_tensor("crit", (4, D), F32,
                            kind="ExternalInput").ap()
    y_d = nc.dram_tensor("y", (BPC, D), F32, kind="ExternalOutput").ap()

    def mmv(ap):
        return ap.bitcast(F32R)

    with tile.TileContext(nc) as tc:
        with (
            tc.tile_pool(name="singles", bufs=1) as singles,
            tc.tile_pool(name="psd", bufs=2, space="PSUM") as psd,
            tc.tile_pool(name="psv", bufs=1, space="PSUM") as psv,
            tc.tile_pool(name="r4p", bufs=R4B) as r4p,
            tc.tile_pool(name="p4p", bufs=P4B) as p4p,
            tc.tile_pool(name="absp", bufs=3) as absp,
            tc.tile_pool(name="smalls", bufs=1) as smalls,
        ):
            # ---------------- prologue: DMAs + memsets ----------------
            # d-matmul operands: lhsT [2, BPC*D] = [x_row; ones],
            # rhs [2, NPAIR*D] = [ak bcast; u rows]
            lhs2 = singles.tile([2, BPC * D], F32R, tag="lhs2")
            rhs2 = singles.tile([2, NPAIR * D], F32R, tag="rhs2")

            def flat(dram, r0, n):
                return bass.AP(tensor=dram.tensor,
                               offset=dram.offset + r0 * D,
                               ap=[[0, 1], [1, n * D]]).bitcast(F32R)

            # pair-0 critical path: 2 combined 2-row DMAs in parallel
            def crit2(r0):
                return bass.AP(tensor=crit_d.tensor,
                               offset=crit_d.offset + r0 * D,
                               ap=[[D, 2], [1, D]]).bitcast(F32R)

            nc.sync.dma_start(out=lhs2[0:2, 0:D], in_=crit2(0))
            nc.scalar.dma_start(out=rhs2[0:2, 0:D], in_=crit2(2))
            # remaining pairs: 2 DMAs each, spread over queues
            qs = [nc.sync, nc.scalar, nc.gpsimd]
            for p in range(1, NPAIR):
                q = qs[p % 3]
                q.dma_start(out=rhs2[0:1, p * D:(p + 1) * D],
                            in_=flat(akb_d, p, 1))
                q.dma_start(out=rhs2[1:2, p * D:(p + 1) * D],
                            in_=flat(u8_d, p, 1))
            nc.scalar.dma_start(out=lhs2[0:1, D:BPC * D],
                                in_=flat(xr_d, 1, BPC - 1))
            nc.scalar.dma_start(
                out=lhs2[1:2, D:BPC * D],
                in_=bass.AP(tensor=xr_d.tensor,
                            offset=xr_d.offset + BPC * D,
                            ap=[[0, 1], [0, BPC - 1], [1, D]]).bitcast(F32R))

            # value lhsT, all pairs: [128, NPAIR*3*NT] cols (avx|1|xbf)
            lhsvB = singles.tile([128, NPAIR * 3 * NT], BF16, tag="lhsvB")
            nc.gpsimd.memset(lhsvB, 1.0)
            nc.scalar.dma_start(
                out=bass.AP(tensor=lhsvB.tensor, offset=lhsvB.offset,
                            ap=[list(lhsvB.ap[0]), [3 * NT, NPAIR],
                                [3, NT]]),
                in_=bass.AP(tensor=avx_d.tensor, offset=avx_d.offset,
                            ap=[[1, 128], [D, NPAIR], [128, NT]]))
            nc.scalar.dma_start(
                out=bass.AP(tensor=lhsvB.tensor, offset=lhsvB.offset + 2,
                            ap=[list(lhsvB.ap[0]), [3 * NT, NPAIR],
                                [3, NT]]),
                in_=bass.AP(tensor=xbf8_d.tensor, offset=xbf8_d.offset,
                            ap=[[1, 128], [D, NPAIR], [128, NT]]))

            # epilogue constants
            nbias = singles.tile([128, 1], F32, tag="nbias")
            nc.gpsimd.memset(nbias, -BIAS)
            zbias = singles.tile([128, 1], F32, name="zbias", tag="zbias")
            nc.vector.memset(zbias, 0.0)
            col8 = singles.tile([8, 2], F32, tag="col8")
            nc.gpsimd.dma_start(out=col8, in_=col8_d)
            lhy = singles.tile([8, 2], BF16, tag="lhy")
            nc.gpsimd.dma_start(out=lhy, in_=lhy_d)
            att9 = singles.tile([8, 1024], BF16, tag="att9")
            xr2 = singles.tile([BPC, D], F32, tag="xr2")
            nc.gpsimd.dma_start(out=xr2, in_=xb2_d)
            u8h = singles.tile([8, D], BF16, tag="u8h")
            nc.gpsimd.dma_start(out=u8h, in_=u8h_d)

            # PSUM val tiles (4 pairs each at partitions 0/32/64/96, M=3)
            val0 = psv.tile([128, D], F32, tag="val0")
            val1 = psv.tile([128, D], F32, tag="val1")
            vt = [val0, val1]

            # PE p-state warmup into val1's last-used region
            warm_in = singles.tile([3, 128], F32, tag="warm_in")
            nc.vector.memset(warm_in, 1.0)
            for _ in range(5):
                nc.tensor.matmul(val1[0:128, 896:1024],
                                 mmv(warm_in[0:3, 0:128]),
                                 mmv(warm_in[0:3, 0:128]),
                                 start=True, stop=True, skip_group_check=True)

            # ---------------- main pipeline ----------------
            r4_of = {}
            p4_of = {}
            pend_val = []                  # [(p, jt, p4, slot)]

            def emit_d(p, jt):
                b = PERM[p][0]
                d2 = psd.tile([128, D], F32, name="d2", tag="d2")
                lt = lhs2[0:2, b * D + jt * 128: b * D + (jt + 1) * 128]
                for c in range(2):
                    rt = rhs2[0:2, p * D + c * 512: p * D + (c + 1) * 512]
                    nc.tensor.matmul(d2[:, c * 512:(c + 1) * 512], lt, rt,
                                     start=True, stop=True)
                return d2

            def emit_transform(p, jt, g, slot):
                if g not in r4_of:
                    r4_of[g] = r4p.tile([128, gsz[g] * 1024], FP16, name="r4",
                                        tag="r4")
                r4 = r4_of[g]
                rsl = r4[:, slot * 1024:(slot + 1) * 1024]
                d2 = emit_d(p, jt)
                if (p, jt) in POW_TILES:
                    ab = absp.tile([128, D], FP16, tag="ab")
                    nc.scalar.activation(out=ab, in_=d2, func=AF.Abs,
                                         bias=zbias, scale=1.0)
                    nc.vector.tensor_scalar(out=rsl, in0=ab,
                                            scalar1=1.0 / CLAMP,
                                            scalar2=-1.0,
                                            op0=OP.max, op1=OP.pow)
                else:
                    nc.vector._custom_dve(RECIP_OP, out=rsl, in0=d2,
                                          s0=C0V, s1=C1V, imm2=1.0 / CLAMP)

            def emit_exp(g):
                r4 = r4_of.pop(g)
                p4 = p4p.tile([128, gsz[g] * 1024], BF16, name="p4",
                              tag="p4")
                nc.scalar.activation(out=p4, in_=r4, func=AF.Exp,
                                     bias=nbias, scale=1.0)
                p4_of[g] = p4

            def emit_val(p, jt, p4, slot):
                g8 = p % 4                 # partition group in val tile
                v = vt[p // 4]
                for c in range(2):
                    js = slice(c * 512, (c + 1) * 512)
                    rhs = p4[:, slot * 1024 + c * 512:
                             slot * 1024 + (c + 1) * 512]
                    nc.tensor.matmul(
                        v[32 * g8:32 * g8 + 3, js],
                        lhsvB[:, p * 3 * NT + 3 * jt:p * 3 * NT + 3 * jt + 3], rhs,
                        start=(jt == 0), stop=(jt == NT - 1),
                        skip_group_check=True,
                        tile_position=(0, 32 * g8))

            tiles = [(p, jt) for p in range(NPAIR) for jt in range(NT)]
            NTL = len(tiles)
            gmap = []
            gsz = {}
            for idx in range(NTL):
                if idx >= NTL - 4:
                    g = (NTL - 4) // EXPB + (idx - (NTL - 4))
                    slot = 0
                    gsz[g] = 1
                else:
                    g, slot = divmod(idx, EXPB)
                    gsz[g] = EXPB
                gmap.append((g, slot))
            gbase = {}
            for idx in range(NTL):
                g, slot = gmap[idx]
                if slot == 0:
                    gbase[g] = idx
            for idx, (p, jt) in enumerate(tiles):
                g, slot = gmap[idx]
                emit_transform(p, jt, g, slot)
                if slot == gsz[g] - 1:
                    emit_exp(g)
                    # lag value matmuls two groups behind so the PE queue
                    # never head-blocks on the transform/exp chain
                    if g - VLAG in p4_of:
                        gd = g - VLAG
                        p4 = p4_of.pop(gd)
                        for s in range(gsz[gd]):
                            pp, jj = tiles[gbase[gd] + s]
                            emit_val(pp, jj, p4, s)
            # drain the remaining groups
            for gd in sorted(p4_of):
                p4 = p4_of[gd]
                for s in range(gsz[gd]):
                    pp, jj = tiles[gbase[gd] + s]
                    emit_val(pp, jj, p4, s)
            p4_of.clear()

            # ---------------- epilogue ----------------
            # PSUM cannot source DMA: bounce val tiles to SBUF first, then
            # gather N/Z/X rows (partition stride 32) via SBUF->SBUF DMA.
            sv0 = smalls.tile([99, D], BF16, tag="sv0")
            nc.scalar.activation(out=sv0, in_=val0[0:99, :],
                                 func=AF.Identity, bias=zbias[0:99, :],
                                 scale=1.0)
            sv1 = smalls.tile([99, D], BF16, tag="sv1")
            nc.scalar.activation(out=sv1, in_=val1[0:99, :],
                                 func=AF.Identity, bias=zbias[0:99, :],
                                 scale=1.0)
            sv = [sv0, sv1]

            def gather(rt, tag):
                t8 = smalls.tile([8, D], BF16, name="t8", tag=tag)
                for half in range(2):
                    v = sv[half]
                    pitch = v.ap[0][0]
                    ap = bass.AP(tensor=v.tensor,
                                 offset=v.offset + rt * pitch,
                                 ap=[[32 * pitch, 4], [1, D]])
                    q = nc.sync if half else nc.scalar
                    q.dma_start(out=t8[4 * half:4 * half + 4, :], in_=ap)
                return t8

            z8 = gather(1, "z8")
            n8 = gather(0, "n8")
            x8 = gather(2, "x8")

            # val0 rows (correction heads): full chain on idle Pool,
            # overlapped with the second half of the main pipeline.
            iz0 = smalls.tile([4, D], F32, tag="iz0")
            nc.vector.reciprocal(out=iz0, in_=z8[0:4, :])
            xz0 = smalls.tile([4, D], BF16, tag="xz0")
            nc.gpsimd.tensor_tensor(out=xz0, in0=x8[0:4, :], in1=iz0,
                                    op=OP.mult)
            rho0 = smalls.tile([4, D], BF16, tag="rho0")
            nc.gpsimd.tensor_scalar(out=rho0, in0=z8[0:4, :], scalar1=THETA,
                                    scalar2=col8[0:4, 1:2], op0=OP.is_ge,
                                    op1=OP.mult)
            w10 = smalls.tile([4, D], BF16, tag="w10")
            nc.gpsimd.tensor_scalar(out=w10, in0=xz0, scalar1=col8[0:4, 0:1],
                                    scalar2=None, op0=OP.mult)
            w80 = smalls.tile([4, D], BF16, tag="w80")
            nc.gpsimd.tensor_tensor(out=w80, in0=w10, in1=u8h[0:4, :],
                                    op=OP.add)
            corr0 = smalls.tile([4, D], BF16, tag="corr0")
            nc.gpsimd.tensor_tensor(out=corr0, in0=w80, in1=rho0, op=OP.mult)
            # tail: full-range recip+mult (partition base must be 0),
            # then subtract the precomputed correction on rows 0-3
            iz8 = smalls.tile([8, D], F32, tag="iz8")
            nc.vector.reciprocal(out=iz8, in_=z8)
            nc.vector.tensor_tensor(out=att9, in0=n8, in1=iz8, op=OP.mult)
            nc.vector.tensor_tensor(out=att9[0:4, :], in0=att9[0:4, :],
                                    in1=corr0, op=OP.subtract)

            # y = lhy^T @ att9 (bf16) -> PSUM, then exact fp32 x-residual
            for c in range(2):
                js = slice(c * 512, (c + 1) * 512)
                nc.tensor.matmul(val0[0:BPC, js], lhy, att9[:, js],
                                 start=True, stop=True,
                                 skip_group_check=True)
            ysb = smalls.tile([BPC, D], F32, tag="ysb")
            for c in range(2):
                js = slice(c * 512, (c + 1) * 512)
                nc.vector.tensor_tensor(out=ysb[:, js],
                                        in0=val0[0:BPC, js],
                                        in1=xr2[:, js], op=OP.add)
                nc.sync.dma_start(out=y_d[:, js], in_=ysb[:, js])

    nc.compile()
    return nc


_NC_CACHE = {}


def _get_nc():
    if "nc" not in _NC_CACHE:
        _NC_CACHE["nc"] = build_bass()
    return _NC_CACHE["nc"]


def _round_f32r(a):
    a = np.ascontiguousarray(np.asarray(a, np.float32))
    ai = a.view(np.int32)
    out = ((ai + np.int32(1 << 9)) >> 10) << 10
    return out.view(np.float32)


def kernel(**inputs) -> np.ndarray:
    x = np.ascontiguousarray(np.asarray(inputs["x"], dtype=np.float32))
    aq = np.asarray(inputs["alpha_q"], np.float32)[0]
    bq = np.asarray(inputs["beta_q"], np.float32)[0]
    ak = np.asarray(inputs["alpha_k"], np.float32)[0]
    bk = np.asarray(inputs["beta_k"], np.float32)[0]
    av = np.asarray(inputs["alpha_v"], np.float32)[0]
    bv = np.asarray(inputs["beta_v"], np.float32)[0]

    akr = _round_f32r(ak.reshape(1, H))
    bvs = float((bv * np.float32(ISH)).sum())
    ish = np.float32(ISH)

    nc = _get_nc()
    in_maps = []
    for c in range(N_CORES):
        xs = x[c * BPC:(c + 1) * BPC]
        xr = np.concatenate([_round_f32r(xs), np.ones((1, D), np.float32)],
                            axis=0)
        u8 = np.empty((NPAIR, D), np.float32)
        avx = np.empty((NPAIR, D), ml_dtypes.bfloat16)
        col8 = np.empty((8, 2), np.float32)
        for p in range(NPAIR):
            b, h = PERM[p]
            u8[p] = _round_f32r(np.float32(bk[h] - bq[h]) - aq[h] * xs[b])
            avx[p] = (xs[b] * np.float32(av[h] * ish)).astype(
                ml_dtypes.bfloat16)
            col8[p, 0] = akr[0, h]
            col8[p, 1] = float(av[h] * ish / ak[h])
        lhy = np.zeros((8, 2), np.float32)
        for p in range(NPAIR):
            lhy[p, PERM[p][0]] = 1.0
        akb = np.repeat(akr[0][np.array([PERM[p][1] for p in range(NPAIR)])],
                        D).reshape(NPAIR, D).astype(np.float32)
        m = {
            "xr": np.ascontiguousarray(xr),
            "u8": np.ascontiguousarray(u8),
            "akb": np.ascontiguousarray(akb),
            "avx": np.ascontiguousarray(avx),
            "xbf8": np.ascontiguousarray(
                xs.astype(ml_dtypes.bfloat16)[
                    np.array([PERM[p][0] for p in range(NPAIR)])]),
            "col8": col8,
            "lhy": lhy.astype(ml_dtypes.bfloat16),
            "xb2": np.ascontiguousarray(xs + np.float32(bvs)),
            "crit": np.ascontiguousarray(np.stack([
                xr[0], np.ones(D, np.float32), akb[0], u8[0]])),
            "u8h": np.ascontiguousarray(u8.astype(ml_dtypes.bfloat16)),
        }
        in_maps.append(m)
    res = run_bass_kernel_spmd(nc, in_maps, core_ids=list(range(N_CORES)))
    return np.concatenate([r["y"] for r in res.results], axis=0)


if __name__ == "__main__":
    rng = np.random.default_rng(0)
    demo = {
        "x": rng.standard_normal((B, D), dtype=np.float32),
        "alpha_q": rng.random((1, H), dtype=np.float32),
        "beta_q": np.zeros((1, H), np.float32),
        "alpha_k": rng.random((1, H), dtype=np.float32),
        "beta_k": np.zeros((1, H), np.float32),
        "alpha_v": rng.random((1, H), dtype=np.float32),
        "beta_v": np.zeros((1, H), np.float32),
    }
    out = kernel(**demo)
    print("kernel output", out.shape, out.dtype)


# revision 28
# speedup vs baseline: 1.0077x; 1.0077x over previous
"""Trainium2 Bass kernel for nn_FLAttention (sparse_attention) — j-layout.

Math (per batch b, head h), q = aq*x+bq, k = ak*x+bk, v = av*x+bv:
  S[i,j] = 1/(|k_j - q_i| + eps);  P = softmax_j(S);  att = P v / sqrt(H)
  out = x + sum_h att

Key idea vs the i-layout baseline: put KEYS j on partitions and queries i
on the free dim.  Then Z_i = sum_j p_ji and N_i = sum_j p_ji*avx_j and
X_i = sum_j p_ji*x_j are PE matmuls (lhsT = [avx; 1; x], M=3) instead of
Pool/DVE elementwise passes.  Softmax stability without a row max: clamp
a = max(|d|, 1/C) with C=160 and bias p = exp(r - 85); all terms live in
[e^-85, e^75] (fp32/bf16 safe) and ratios below the clamp are exact.
Rows where the clamp ties multiple keys are repaired in the epilogue:
for tied rows  sum_j p (v_j - (av/ak) d_j)/Z = v(d=0)  identically, so
att_corr = att - rho*(av/ak)*(ak*X/Z + u)  with rho = [Z > e^75.2].

Per 128x1024 tile: PE d-matmul (K=2, f32r, two N=512 chunks into a
2-deep PSUM rotation), DVE custom op CLAMP_RECIP (abs+clamp+1NR recip,
8 ALU stages, fp16 out), ACT exp batched EXPB tiles/instr (bf16 p), PE
value matmul (M=3 = [avISH*x; 1; x] columns, bf16, PSUM-accumulated at
partition offsets 32g).  TRN2 notes: GPSIMD/Pool cannot touch PSUM and
has no divide/pow; compute operands need 32-aligned partition bases.
Epilogue: val tiles -> SBUF via ACT Identity (PSUM-capable), strided
gathers of N/Z/X rows, the v*-correction chain for the small-ak heads
(ordered into val tile 0) on the otherwise-idle Pool overlapped with
the main pipeline, a short DVE tail (reciprocal + N*iZ - corr), one
K=8 bf16 matmul summing heads, and an exact fp32 x+bvs residual add.

Sharding: data-parallel over batch: B=16 -> 2 batches per core, 8 cores.
"""
import numpy as np
import ml_dtypes

import concourse.bass as bass
import concourse.bacc as bacc
import concourse.mybir as mybir
import concourse.tile as tile
from concourse.bass_utils import run_bass_kernel_spmd

B, D, H = 16, 1024, 4
N_CORES = 8
BPC = B // N_CORES          # batches per core
NPAIR = BPC * H             # (b,h) pairs per core
NT = D // 128               # j-tiles per pair

F32 = mybir.dt.float32
F32R = mybir.dt.float32r
BF16 = mybir.dt.bfloat16
FP16 = mybir.dt.float16
OP = mybir.AluOpType
AF = mybir.ActivationFunctionType

CLAMP = 160.0               # s-clamp: a = max(|d|, 1/CLAMP)
BIAS = 85.0                 # p = exp(r - BIAS)
THETA = float(np.exp(75.2))  # correction trigger: Z > THETA
ISH = float(1.0 / np.sqrt(np.float32(H)))
# pair order: correction heads (0,1) of both batches first (-> val tile 0),
# then heads 2,3 (-> val tile 1, no epilogue correction needed)
PERM = [(0, 0), (0, 1), (1, 0), (1, 1), (0, 2), (0, 3), (1, 2), (1, 3)]
EXPB = 2                    # tiles per batched exp instruction
R4B = 8                     # r4 pool depth
P4B = 8                     # p4 pool depth
VLAG = 2                    # val-matmul group lag

# (pair, jt) tiles whose abs+clamp runs on Pool (divide then on DVE):
# tiles whose transform goes ACT-abs -> Pool (max, then pow -1):
POW_TILES = frozenset((p, jt) for p in range(8) for jt in (2, 6)
                      if (p, jt) not in ((0, 2), (1, 2), (0, 6), (1, 6),
                                         (2, 2), (3, 2)))

# ---------------- custom DVE op: r = NR1(1/max(|d|, C2)) --------------------
from concourse.dve_spec import (Spec, Src0, C0, C1, C2, Zero, Bin, AluOp,
                                 lower)
from concourse.dve_uop import DveOpSpec
from concourse.dve_ops import DveOp, RECIP_APPROX_FAST_CONSTS
import concourse.dve_ops as dve_ops

RECIP_NAME = "CLAMP_RECIP_ANT"
C0V = RECIP_APPROX_FAST_CONSTS["s0"]
C1V = RECIP_APPROX_FAST_CONSTS["s1"]


def _clamp_recip_ref(in0, in1, c0, c1, c2):
    # a = max(d, -d, c2); 1-NR reciprocal from BITWISE_NOT exponent seed
    x = np.maximum(in0.astype(np.float32),
                   (np.float32(0.0) - in0).astype(np.float32))
    a = np.maximum(x, np.float32(c2))
    not_a = (~a.view(np.int32)).view(np.float32)
    y0 = not_a * np.float32(c0)
    y1 = (y0 * (np.float32(c1) - a * y0)).astype(np.float32)
    return y1


def _register_recip_op():
    if RECIP_NAME in dve_ops._SUB_OPCODE_FOR_NAME:
        for o in dve_ops.OPS:
            if o.name == RECIP_NAME:
                return o
    t = Bin(AluOp.SUBTRACT, Zero, Src0)
    x = Bin(AluOp.MAX, Src0, t)
    a = Bin(AluOp.MAX, x, C2)
    nx = Bin(AluOp.BITWISE_NOT, a, a)
    y0 = Bin(AluOp.MULTIPLY, nx, C0)
    y1 = Bin(AluOp.MULTIPLY, y0,
             Bin(AluOp.SUBTRACT, C1, Bin(AluOp.MULTIPLY, a, y0)))
    spec = Spec(body=y1, reference=_clamp_recip_ref)
    row = max(dve_ops._SUB_OPCODE_FOR_NAME.values()) + 1
    assert row < 0x20
    dve_ops._SUB_OPCODE_FOR_NAME[RECIP_NAME] = row
    shas = {}
    for ver in ("v3", "v4"):
        s = DveOpSpec(name=RECIP_NAME, opcode=row, uops=lower(spec, ver=ver),
                      rd1_en=False)
        shas[ver] = s.sha(ver)
    op = DveOp(RECIP_NAME, spec, subdim=False, uops_sha=shas)
    dve_ops.OPS.append(op)
    dve_ops.CUSTOM_DVE_SPECS[RECIP_NAME] = spec
    return op


RECIP_OP = _register_recip_op()


def build_bass():
    nc = bacc.Bacc(
        "TRN2",
        target_bir_lowering=False,
        debug=False,
        enable_asserts=False,
        num_devices=N_CORES,
    )
    # host-prepped inputs (see kernel()):
    # xr  = [f32r-rounded x rows (BPC); ones row]
    # u8  = per-pair rows cc_h - aq_h*x_b (f32r)   [also correction input]
    # akr = f32r ak values
    # avx = per-pair bf16 rows av_h*ISH*x_b ; xbf = bf16 x rows
    # col8 = [ak(8); cav(8)] per-pair columns; lhy = y-matmul lhsT [11,2]
    xr_d = nc.dram_tensor("xr", (BPC + 1, D), F32, kind="ExternalInput").ap()
    u8_d = nc.dram_tensor("u8", (NPAIR, D), F32, kind="ExternalInput").ap()
    akb_d = nc.dram_tensor("akb", (NPAIR, D), F32, kind="ExternalInput").ap()
    avx_d = nc.dram_tensor("avx", (NPAIR, D), BF16, kind="ExternalInput").ap()
    xbf8_d = nc.dram_tensor("xbf8", (NPAIR, D), BF16,
                            kind="ExternalInput").ap()
    col8_d = nc.dram_tensor("col8", (8, 2), F32, kind="ExternalInput").ap()
    lhy_d = nc.dram_tensor("lhy", (8, 2), BF16, kind="ExternalInput").ap()
    u8h_d = nc.dram_tensor("u8h", (NPAIR, D), BF16,
                           kind="ExternalInput").ap()
    xb2_d = nc.dram_tensor("xb2", (BPC, D), F32,
                           kind="ExternalInput").ap()
    crit_d = nc.dram_tensor("crit", (4, D), F32,
                            kind="ExternalInput").ap()
    y_d = nc.dram_tensor("y", (BPC, D), F32, kind="ExternalOutput").ap()

    def mmv(ap):
        return ap.bitcast(F32R)

    with tile.TileContext(nc) as tc:
        with (
            tc.tile_pool(name="singles", bufs=1) as singles,
            tc.tile_pool(name="psd", bufs=2, space="PSUM") as psd,
            tc.tile_pool(name="psv", bufs=1, space="PSUM") as psv,
            tc.tile_pool(name="r4p", bufs=R4B) as r4p,
            tc.tile_pool(name="p4p", bufs=P4B) as p4p,
            tc.tile_pool(name="absp", bufs=3) as absp,
            tc.tile_pool(name="smalls", bufs=1) as smalls,
        ):
            # ---------------- prologue: DMAs + memsets ----------------
            # d-matmul operands: lhsT [2, BPC*D] = [x_row; ones],
            # rhs [2, NPAIR*D] = [ak bcast; u rows]
            lhs2 = singles.tile([2, BPC * D], F32R, tag="lhs2")
            rhs2 = singles.tile([2, NPAIR * D], F32R, tag="rhs2")

            def flat(dram, r0, n):
                return bass.AP(tensor=dram.tensor,
                               offset=dram.offset + r0 * D,
                               ap=[[0, 1], [1, n * D]]).bitcast(F32R)

            # pair-0 critical path: 2 combined 2-row DMAs in parallel
            def crit2(r0):
                return bass.AP(tensor=crit_d.tensor,
                               offset=crit_d.offset + r0 * D,
                               ap=[[D, 2], [1, D]]).bitcast(F32R)

            nc.sync.dma_start(out=lhs2[0:2, 0:D], in_=crit2(0))
            nc.scalar.dma_start(out=rhs2[0:2, 0:D], in_=crit2(2))
            # remaining pairs: 2 DMAs each, spread over queues
            qs = [nc.sync, nc.scalar, nc.gpsimd]
            for p in range(1, NPAIR):
                q = qs[p % 3]
                q.dma_start(out=rhs2[0:1, p * D:(p + 1) * D],
                            in_=flat(akb_d, p, 1))
                q.dma_start(out=rhs2[1:2, p * D:(p + 1) * D],
                            in_=flat(u8_d, p, 1))
            nc.scalar.dma_start(out=lhs2[0:1, D:BPC * D],
                                in_=flat(xr_d, 1, BPC - 1))
            nc.scalar.dma_start(
                out=lhs2[1:2, D:BPC * D],
                in_=bass.AP(tensor=xr_d.tensor,
                            offset=xr_d.offset + BPC * D,
                            ap=[[0, 1], [0, BPC - 1], [1, D]]).bitcast(F32R))

            # value lhsT, all pairs: [128, NPAIR*3*NT] cols (avx|1|xbf)
            lhsvB = singles.tile([128, NPAIR * 3 * NT], BF16, tag="lhsvB")
            nc.gpsimd.memset(lhsvB, 1.0)
            nc.scalar.dma_start(
                out=bass.AP(tensor=lhsvB.tensor, offset=lhsvB.offset,
                            ap=[list(lhsvB.ap[0]), [3 * NT, NPAIR],
                                [3, NT]]),
                in_=bass.AP(tensor=avx_d.tensor, offset=avx_d.offset,
                            ap=[[1, 128], [D, NPAIR], [128, NT]]))
            nc.scalar.dma_start(
                out=bass.AP(tensor=lhsvB.tensor, offset=lhsvB.offset + 2,
                            ap=[list(lhsvB.ap[0]), [3 * NT, NPAIR],
                                [3, NT]]),
                in_=bass.AP(tensor=xbf8_d.tensor, offset=xbf8_d.offset,
                            ap=[[1, 128], [D, NPAIR], [128, NT]]))

            # epilogue constants
            nbias = singles.tile([128, 1], F32, tag="nbias")
            nc.gpsimd.memset(nbias, -BIAS)
            zbias = singles.tile([128, 1], F32, name="zbias", tag="zbias")
            nc.vector.memset(zbias, 0.0)
            col8 = singles.tile([8, 2], F32, tag="col8")
            nc.gpsimd.dma_start(out=col8, in_=col8_d)
            lhy = singles.tile([8, 2], BF16, tag="lhy")
            nc.gpsimd.dma_start(out=lhy, in_=lhy_d)
            att9 = singles.tile([8, 1024], BF16, tag="att9")
            xr2 = singles.tile([BPC, D], F32, tag="xr2")
            nc.gpsimd.dma_start(out=xr2, in_=xb2_d)
            u8h = singles.tile([8, D], BF16, tag="u8h")
            nc.gpsimd.dma_start(out=u8h, in_=u8h_d)

            # PSUM val tiles (4 pairs each at partitions 0/32/64/96, M=3)
            val0 = psv.tile([128, D], F32, tag="val0")
            val1 = psv.tile([128, D], F32, tag="val1")
            vt = [val0, val1]

            # PE p-state warmup into val1's last-used region
            warm_in = singles.tile([3, 128], F32, tag="warm_in")
            nc.vector.memset(warm_in, 1.0)
            for _ in range(5):
                nc.tensor.matmul(val1[0:128, 896:1024],
                                 mmv(warm_in[0:3, 0:128]),
                                 mmv(warm_in[0:3, 0:128]),
                                 start=True, stop=True, skip_group_check=True)

            # ---------------- main pipeline ----------------
            r4_of = {}
            p4_of = {}
            pend_val = []                  # [(p, jt, p4, slot)]

            def emit_d(p, jt):
                b = PERM[p][0]
                d2 = psd.tile([128, D], F32, name="d2", tag="d2")
                lt = lhs2[0:2, b * D + jt * 128: b * D + (jt + 1) * 128]
                for c in range(2):
                    rt = rhs2[0:2, p * D + c * 512: p * D + (c + 1) * 512]
                    nc.tensor.matmul(d2[:, c * 512:(c + 1) * 512], lt, rt,
                                     start=True, stop=True)
                return d2

            def emit_transform(p, jt, g, slot):
                if g not in r4_of:
                    r4_of[g] = r4p.tile([128, gsz[g] * 1024], FP16, name="r4",
                                        tag="r4")
                r4 = r4_of[g]
                rsl = r4[:, slot * 1024:(slot + 1) * 1024]
                d2 = emit_d(p, jt)
                if (p, jt) in POW_TILES:
                    ab = absp.tile([128, D], FP16, tag="ab")
                    nc.scalar.activation(out=ab, in_=d2, func=AF.Abs,
                                         bias=zbias, scale=1.0)
                    nc.vector.tensor_scalar(out=rsl, in0=ab,
                                            scalar1=1.0 / CLAMP,
                                            scalar2=-1.0,
                                            op0=OP.max, op1=OP.pow)
                else:
                    nc.vector._custom_dve(RECIP_OP, out=rsl, in0=d2,
                                          s0=C0V, s1=C1V, imm2=1.0 / CLAMP)

            def emit_exp(g):
                r4 = r4_of.pop(g)
                p4 = p4p.tile([128, gsz[g] * 1024], BF16, name="p4",
                              tag="p4")
                nc.scalar.activation(out=p4, in_=r4, func=AF.Exp,
                                     bias=nbias, scale=1.0)
                p4_of[g] = p4

            def emit_val(p, jt, p4, slot):
                g8 = p % 4                 # partition group in val tile
                v = vt[p // 4]
                for c in range(2):
                    js = slice(c * 512, (c + 1) * 512)
                    rhs = p4[:, slot * 1024 + c * 512:
                             slot * 1024 + (c + 1) * 512]
                    nc.tensor.matmul(
                        v[32 * g8:32 * g8 + 3, js],
                        lhsvB[:, p * 3 * NT + 3 * jt:p * 3 * NT + 3 * jt + 3], rhs,
                        start=(jt == 0), stop=(jt == NT - 1),
                        skip_group_check=True,
                        tile_position=(0, 32 * g8))

            tiles = [(p, jt) for p in range(NPAIR) for jt in range(NT)]
            NTL = len(tiles)
            gmap = []
            gsz = {}
            for idx in range(NTL):
                if idx >= NTL - 4:
                    g = (NTL - 4) // EXPB + (idx - (NTL - 4))
                    slot = 0
                    gsz[g] = 1
                else:
                    g, slot = divmod(idx, EXPB)
                    gsz[g] = EXPB
                gmap.append((g, slot))
            gbase = {}
            for idx in range(NTL):
                g, slot = gmap[idx]
                if slot == 0:
                    gbase[g] = idx
            for idx, (p, jt) in enumerate(tiles):
                g, slot = gmap[idx]
                emit_transform(p, jt, g, slot)
                if slot == gsz[g] - 1:
                    emit_exp(g)
                    # lag value matmuls two groups behind so the PE queue
                    # never head-blocks on the transform/exp chain
                    if g - VLAG in p4_of:
                        gd = g - VLAG
                        p4 = p4_of.pop(gd)
                        for s in range(gsz[gd]):
                            pp, jj = tiles[gbase[gd] + s]
                            emit_val(pp, jj, p4, s)
            # drain the remaining groups
            for gd in sorted(p4_of):
                p4 = p4_of[gd]
                for s in range(gsz[gd]):
                    pp, jj = tiles[gbase[gd] + s]
                    emit_val(pp, jj, p4, s)
            p4_of.clear()

            # ---------------- epilogue ----------------
            # PSUM cannot source DMA: bounce val tiles to SBUF first, then
            # gather N/Z/X rows (partition stride 32) via SBUF->SBUF DMA.
            sv0 = smalls.tile([99, D], BF16, tag="sv0")
            nc.scalar.activation(out=sv0, in_=val0[0:99, :],
                                 func=AF.Identity, bias=zbias[0:99, :],
                                 scale=1.0)
            sv1 = smalls.tile([99, D], BF16, tag="sv1")
            nc.scalar.activation(out=sv1, in_=val1[0:99, :],
                                 func=AF.Identity, bias=zbias[0:99, :],
                                 scale=1.0)
            sv = [sv0, sv1]

            def gather(rt, tag):
                t8 = smalls.tile([8, D], BF16, name="t8", tag=tag)
                for half in range(2):
                    v = sv[half]
                    pitch = v.ap[0][0]
                    ap = bass.AP(tensor=v.tensor,
                                 offset=v.offset + rt * pitch,
                                 ap=[[32 * pitch, 4], [1, D]])
                    q = nc.sync if half else nc.scalar
                    q.dma_start(out=t8[4 * half:4 * half + 4, :], in_=ap)
                return t8

            z8 = gather(1, "z8")
            n8 = gather(0, "n8")
            x8 = gather(2, "x8")

            # val0 rows (correction heads): full chain on idle Pool,
            # overlapped with the second half of the main pipeline.
            iz0 = smalls.tile([4, D], F32, tag="iz0")
            nc.vector.reciprocal(out=iz0, in_=z8[0:4, :])
            xz0 = smalls.tile([4, D], BF16, tag="xz0")
            nc.gpsimd.tensor_tensor(out=xz0, in0=x8[0:4, :], in1=iz0,
                                    op=OP.mult)
            rho0 = smalls.tile([4, D], BF16, tag="rho0")
            nc.gpsimd.tensor_scalar(out=rho0, in0=z8[0:4, :], scalar1=THETA,
                                    scalar2=col8[0:4, 1:2], op0=OP.is_ge,
                                    op1=OP.mult)
            w10 = smalls.tile([4, D], BF16, tag="w10")
            nc.gpsimd.tensor_scalar(out=w10, in0=xz0, scalar1=col8[0:4, 0:1],
                                    scalar2=None, op0=OP.mult)
            w80 = smalls.tile([4, D], BF16, tag="w80")
            nc.gpsimd.tensor_tensor(out=w80, in0=w10, in1=u8h[0:4, :],
                                    op=OP.add)
            corr0 = smalls.tile([4, D], BF16, tag="corr0")
            nc.gpsimd.tensor_tensor(out=corr0, in0=w80, in1=rho0, op=OP.mult)
            # tail: full-range recip+mult (partition base must be 0),
            # then subtract the precomputed correction on rows 0-3
            iz8 = smalls.tile([8, D], F32, tag="iz8")
            nc.vector.reciprocal(out=iz8, in_=z8)
            nc.vector.tensor_tensor(out=att9, in0=n8, in1=iz8, op=OP.mult)
            nc.vector.tensor_tensor(out=att9[0:4, :], in0=att9[0:4, :],
                                    in1=corr0, op=OP.subtract)

            # y = lhy^T @ att9 (bf16) -> PSUM, then exact fp32 x-residual
            for c in range(2):
                js = slice(c * 512, (c + 1) * 512)
                nc.tensor.matmul(val0[0:BPC, js], lhy, att9[:, js],
                                 start=True, stop=True,
                                 skip_group_check=True)
            ysb = smalls.tile([BPC, D], F32, tag="ysb")
            for c in range(2):
                js = slice(c * 512, (c + 1) * 512)
                nc.vector.tensor_tensor(out=ysb[:, js],
                                        in0=val0[0:BPC, js],
                                        in1=xr2[:, js], op=OP.add)
                nc.sync.dma_start(out=y_d[:, js], in_=ysb[:, js])

    nc.compile()
    return nc


_NC_CACHE = {}


def _get_nc():
    if "nc" not in _NC_CACHE:
        _NC_CACHE["nc"] = build_bass()
    return _NC_CACHE["nc"]


def _round_f32r(a):
    a = np.ascontiguousarray(np.asarray(a, np.float32))
    ai = a.view(np.int32)
    out = ((ai + np.int32(1 << 9)) >> 10) << 10
    return out.view(np.float32)


def kernel(**inputs) -> np.ndarray:
    x = np.ascontiguousarray(np.asarray(inputs["x"], dtype=np.float32))
    aq = np.asarray(inputs["alpha_q"], np.float32)[0]
    bq = np.asarray(inputs["beta_q"], np.float32)[0]
    ak = np.asarray(inputs["alpha_k"], np.float32)[0]
    bk = np.asarray(inputs["beta_k"], np.float32)[0]
    av = np.asarray(inputs["alpha_v"], np.float32)[0]
    bv = np.asarray(inputs["beta_v"], np.float32)[0]

    akr = _round_f32r(ak.reshape(1, H))
    bvs = float((bv * np.float32(ISH)).sum())
    ish = np.float32(ISH)

    nc = _get_nc()
    in_maps = []
    for c in range(N_CORES):
        xs = x[c * BPC:(c + 1) * BPC]
        xr = np.concatenate([_round_f32r(xs), np.ones((1, D), np.float32)],
                            axis=0)
        u8 = np.empty((NPAIR, D), np.float32)
        avx = np.empty((NPAIR, D), ml_dtypes.bfloat16)
        col8 = np.empty((8, 2), np.float32)
        for p in range(NPAIR):
            b, h = PERM[p]
            u8[p] = _round_f32r(np.float32(bk[h] - bq[h]) - aq[h] * xs[b])
            avx[p] = (xs[b] * np.float32(av[h] * ish)).astype(
                ml_dtypes.bfloat16)
            col8[p, 0] = akr[0, h]
            col8[p, 1] = float(av[h] * ish / ak[h])
        lhy = np.zeros((8, 2), np.float32)
        for p in range(NPAIR):
            lhy[p, PERM[p][0]] = 1.0
        akb = np.repeat(akr[0][np.array([PERM[p][1] for p in range(NPAIR)])],
                        D).reshape(NPAIR, D).astype(np.float32)
        m = {
            "xr": np.ascontiguousarray(xr),
            "u8": np.ascontiguousarray(u8),
            "akb": np.ascontiguousarray(akb),
            "avx": np.ascontiguousarray(avx),
            "xbf8": np.ascontiguousarray(
                xs.astype(ml_dtypes.bfloat16)[
                    np.array([PERM[p][0] for p in range(NPAIR)])]),
            "col8": col8,
            "lhy": lhy.astype(ml_dtypes.bfloat16),
            "xb2": np.ascontiguousarray(xs + np.float32(bvs)),
            "crit": np.ascontiguousarray(np.stack([
                xr[0], np.ones(D, np.float32), akb[0], u8[0]])),
            "u8h": np.ascontiguousarray(u8.astype(ml_dtypes.bfloat16)),
        }
        in_maps.append(m)
    res = run_bass_kernel_spmd(nc, in_maps, core_ids=list(range(N_CORES)))
    return np.concatenate([r["y"] for r in res.results], axis=0)


if __name__ == "__main__":
    rng = np.random.default_rng(0)
    demo = {
        "x": rng.standard_normal((B, D), dtype=np.float32),
        "alpha_q": rng.random((1, H), dtype=np.float32),
        "beta_q": np.zeros((1, H), np.float32),
        "alpha_k": rng.random((1, H), dtype=np.float32),
        "beta_k": np.zeros((1, H), np.float32),
        "alpha_v": rng.random((1, H), dtype=np.float32),
        "beta_v": np.zeros((1, H), np.float32),
    }
    out = kernel(**demo)
    print("kernel output", out.shape, out.dtype)
